# revision 28
# baseline (speedup 1.0000x reference)
"""AttentionGCNLayer Trainium2 kernel (final).

Per-sample computation (B=8 samples -> 8 NeuronCores, data-parallel):
  identity = x @ W_it + b_it
  gcn      = relu(adj @ (x @ W_g + b_g))
  h        = LN1(identity + gcn)
  attn     = MHSA(h)  (8 heads, D=32)
  out      = LN2(h + attn)

Design:
  - Host-side weight folding: LN1's gamma folds into W_q/W_k/W_v rows; the
    k-bias drops entirely (softmax is invariant to per-query shifts); the
    v-bias folds into the output-projection bias (softmax rows sum to 1).
    All weights pre-cast to bf16 on the host -> every matmul/transpose is
    single-pass bf16.
  - Softmax exp split across ScalarE (table exp) and VectorE (Schraudolph
    bit-trick: i16 = round(score * 128*scale/ln2 + const), bitcast i16->bf16
    ~= exp(score*scale)). Softmax normalization cancels the approximation's
    bias; end-to-end error ~2.7e-3 (tolerance 2e-2).
  - The per-chunk pre-phase loop emits hT/qkv for chunks PAIRED and LAGGED
    behind the LN1 DVE chain, so the PE never head-of-line blocks on the
    LN statistics; LN1 rsqrt runs one Newton step on chunk pairs.
  - Attention runs a flat slot schedule over all (token-half, head-group,
    k-chunk) slots: attnV+den of slot i-3 interleave with scores+exp of
    slot i, so the PE always has streaming work regardless of exp latency
    and the post-scores drain tail is only 3 slots.
  - adj and x ship as bf16 from the host (validated: no measurable error
    change), halving the dominant DMA traffic and removing all on-device
    input casts.
  - Projection + LN2 + store for the first token half drain while the
    second half's attention streams; the second half is the only tail.
"""

import sys

sys.path.insert(0, "/opt/trn_rl_repo")

import numpy as np

import concourse.bass as bass
import concourse.tile as tile
from concourse import bacc, mybir
from concourse.bass_utils import run_bass_kernel_spmd
from concourse.masks import make_identity

F32 = mybir.dt.float32
BF16 = mybir.dt.bfloat16
I16 = mybir.dt.int16
I32 = mybir.dt.int32
AF = mybir.ActivationFunctionType
ALU = mybir.AluOpType

B, N, CI, CO, H, D = 8, 1024, 128, 256, 8, 32
P = 128
MT = N // P  # 8 token chunks
EPS = 1e-5
SCALE = float(1.0 / np.sqrt(np.float32(D)))
NCORES = 8
MAGIC_P1 = 0x5F3759DF + 1  # quake rsqrt magic + 1 (for the ~t + (M+1) form)

# Schraudolph constants: bf16 bit pattern of exp(scale*x) via int16 affine.
EXP_A = float(SCALE * 128.0 / np.log(2.0))
EXP_B = float(127 * 128 - 9.0)

# which exp slots go to the DVE: (tp == 1) and k in this set (per group)
DVE_EXP_KS = (0, 1, 2, 4, 5, 6)


def _rsqrt_dve(nc, pool, var_ap, out_ap, consts, n, tag, newton=2):
    """out = 1/sqrt(var + eps) on VectorE only, batched over [128, n].

    Quake bit-trick seed + `newton` Newton iterations (1 iter ~0.17% rel
    err, 2 iters ~5e-6). Keeps ScalarE free of Ln/Sqrt so its activation
    table never switches off Exp.
    """
    eps_sb, sh1_i, neg1_i, magic_i = consts
    xe = pool.tile([P, n], F32, tag=f"rs_xe{tag}")
    nc.vector.tensor_scalar_add(xe, var_ap, eps_sb)
    y = pool.tile([P, n], F32, tag=f"rs_y{tag}")
    ti = pool.tile([P, n], I32, tag=f"rs_ti{tag}")
    # ~(x >> 1)
    nc.vector.tensor_scalar(
        out=ti, in0=xe.bitcast(I32), scalar1=sh1_i, scalar2=neg1_i,
        op0=ALU.logical_shift_right, op1=ALU.bitwise_xor)
    # + (MAGIC+1)  ==  MAGIC - (x >> 1)
    nc.vector.tensor_tensor(
        out=y.bitcast(I32), in0=ti, in1=magic_i.to_broadcast((P, n)), op=ALU.add)
    h = pool.tile([P, n], F32, tag=f"rs_h{tag}")
    nc.vector.tensor_scalar_mul(h, xe, 0.5)
    t2 = pool.tile([P, n], F32, tag=f"rs_t2{tag}")
    for _ in range(newton):
        nc.vector.tensor_mul(t2, y, y)
        nc.vector.tensor_mul(t2, t2, h)
        nc.vector.tensor_scalar(
            out=t2, in0=t2, scalar1=-1.0, scalar2=1.5, op0=ALU.mult, op1=ALU.add)
        nc.vector.tensor_mul(y, y, t2)
    nc.vector.tensor_copy(out_ap, y)


def build_bass(trivial1=True, trivial2=True):
    nc = bacc.Bacc()

    x_d = nc.dram_tensor("x", (N, CI), BF16, kind="ExternalInput")
    adj_d = nc.dram_tensor("adj", (N, N), BF16, kind="ExternalInput")
    wit_d = nc.dram_tensor("wit", (CI, CO), BF16, kind="ExternalInput")
    wg_d = nc.dram_tensor("wg", (CI, CO), BF16, kind="ExternalInput")
    wq_d = nc.dram_tensor("wq", (CO, CO), BF16, kind="ExternalInput")
    wk_d = nc.dram_tensor("wk", (CO, CO), BF16, kind="ExternalInput")
    wv_d = nc.dram_tensor("wv", (CO, CO), BF16, kind="ExternalInput")
    wo_d = nc.dram_tensor("wo", (CO, CO), BF16, kind="ExternalInput")
    bit_d = nc.dram_tensor("bit", (CO,), F32, kind="ExternalInput")
    bg_d = nc.dram_tensor("bg", (CO,), F32, kind="ExternalInput")
    bq_d = nc.dram_tensor("bq", (CO,), BF16, kind="ExternalInput")
    bb2_d = nc.dram_tensor("bb2", (CO,), BF16, kind="ExternalInput")
    if not trivial1:
        g1_d = nc.dram_tensor("g1v", (CO,), F32, kind="ExternalInput")
    if not trivial2:
        g2_d = nc.dram_tensor("g2v", (CO,), F32, kind="ExternalInput")
        be2_d = nc.dram_tensor("be2v", (CO,), F32, kind="ExternalInput")
    out_d = nc.dram_tensor("out", (N, CO), F32, kind="ExternalOutput")

    with tile.TileContext(nc) as tc:
        from contextlib import ExitStack

        with ExitStack() as ctx:
            singles = ctx.enter_context(tc.tile_pool(name="singles", bufs=1))
            stemp = ctx.enter_context(tc.tile_pool(name="stemp", bufs=3))
            ptemp = ctx.enter_context(tc.tile_pool(name="ptemp", bufs=5))
            adj_pool = ctx.enter_context(tc.tile_pool(name="adj", bufs=3))
            adjb_pool = ctx.enter_context(tc.tile_pool(name="adjb", bufs=3))
            adjT_pool = ctx.enter_context(tc.tile_pool(name="adjT", bufs=3))
            expT_pool = ctx.enter_context(tc.tile_pool(name="expT", bufs=20))
            ytile_pool = ctx.enter_context(tc.tile_pool(name="ytile", bufs=2))

            # ---------------- Phase -1: identity + big input DMAs first -----
            identB = singles.tile([P, P], BF16)
            make_identity(nc, identB)
            adj_r = adj_d[:].rearrange("(mt p) k -> p mt k", p=P)
            x_sb = singles.tile([P, MT, CI], BF16)
            nc.gpsimd.dma_start(x_sb, x_d[:].rearrange("(mt p) c -> p mt c", p=P))
            adj_tiles = []
            for m, eng in zip(range(3), (nc.sync, nc.scalar, nc.gpsimd)):
                ab = adj_pool.tile([P, N], BF16, tag="ab")
                eng.dma_start(ab, adj_r[:, m, :])
                adj_tiles.append(ab)

            # ---------------- Phase 0: constants / weights ----------------
            ones_sb = singles.tile([P, D], BF16)
            nc.vector.memset(ones_sb, 1.0)
            ones1 = singles.tile([1, 512], BF16)
            nc.vector.memset(ones1, 1.0)
            eps_sb = singles.tile([P, 1], F32)
            nc.vector.memset(eps_sb, EPS)
            sh1_i = singles.tile([P, 1], I32)
            nc.vector.memset(sh1_i, 1)
            neg1_i = singles.tile([P, 1], I32)
            nc.vector.memset(neg1_i, -1)
            magic_i = singles.tile([P, 1], I32)
            nc.vector.memset(magic_i, MAGIC_P1)
            consts = (eps_sb, sh1_i, neg1_i, magic_i)

            def load_w2(dram, name):  # [256,256] -> [128, 2, 256]
                t = singles.tile([P, 2, CO], BF16, tag=f"w2_{name}")
                nc.gpsimd.dma_start(
                    t, dram[:].rearrange("(ko ki) n -> ki ko n", ki=P))
                return t

            def load_row(dram, name):  # [256] -> [1, 256] single-partition row
                t = singles.tile([1, CO], BF16, tag=f"row_{name}")
                nc.gpsimd.dma_start(t, dram[:].rearrange("(a c) -> a c", a=1))
                return t

            def load_bc(dram, name):  # broadcast along partitions: [128, 256]
                t = singles.tile([P, CO], F32, tag=f"bc_{name}")
                src = dram[:]
                bcast = bass.AP(tensor=src.tensor, offset=src.offset,
                                ap=[[0, P]] + list(src.ap))
                nc.gpsimd.dma_start(out=t, in_=bcast)
                return t

            # gpsimd DMA queue ordered by first use: phase-2/gcn inputs
            # first, attention weights after, output-side last.
            wg_sb = singles.tile([P, CO], BF16)
            nc.gpsimd.dma_start(wg_sb, wg_d[:])
            bg_bc = load_bc(bg_d, "bg")
            wit_sb = singles.tile([P, CO], BF16)
            nc.gpsimd.dma_start(wit_sb, wit_d[:])
            bit_bc = load_bc(bit_d, "bit")
            wq_sb = load_w2(wq_d, "wq")
            wk_sb = load_w2(wk_d, "wk")
            wv_sb = load_w2(wv_d, "wv")
            bq_row = load_row(bq_d, "bq")
            wo_sb = load_w2(wo_d, "wo")
            bb2_row = load_row(bb2_d, "bb2")
            if not trivial1:
                g1_bc = load_bc(g1_d, "g1")
            if not trivial2:
                g2_bc = load_bc(g2_d, "g2")
                be2_bc = load_bc(be2_d, "be2")

            # persistent activations
            x_bf = x_sb                                  # already bf16
            xT_bf = singles.tile([P, MT, P], BF16)       # x^T  [ci, m] bf16
            t_sb = singles.tile([P, MT, CO], BF16)       # x@W_g + b_g [tok, c]
            s_all = singles.tile([P, MT, CO], F32)       # pre-LN1 residual
            h_sb = singles.tile([P, MT, CO], F32)        # normalized hhat
            h_bf = singles.tile([P, MT, CO], BF16)       # hhat bf16 (for h^T)
            mv_all = singles.tile([P, MT, 2], F32)       # LN1 mean/var
            rstd_all = singles.tile([P, MT], F32)        # LN1 rstd
            hT_sb = singles.tile([P, 2, N], BF16)        # hhat^T  [c, tok]
            qT_sb = singles.tile([P, 2, N], BF16)        # q^T     [c, tok]
            kT_sb = singles.tile([P, 2, N], BF16)        # k^T     [c, tok]
            v_sb = singles.tile([P, MT, CO], BF16)       # v       [tok, c]
            outT_sb = singles.tile([P, 2, N], BF16)      # attn-out^T [c, tok]

            with ExitStack() as pre:
                tr_ps = pre.enter_context(
                    tc.tile_pool(name="tr_ps", bufs=2, space="PSUM"))
                htr_ps = pre.enter_context(
                    tc.tile_pool(name="htr_ps", bufs=1, space="PSUM"))
                mm_ps = pre.enter_context(
                    tc.tile_pool(name="mm_ps", bufs=2, space="PSUM"))
                qk_ps = pre.enter_context(
                    tc.tile_pool(name="qk_ps", bufs=1, space="PSUM"))

                # Warm-up transpose so PE observes the gpsimd sem early.
                warm_ps = tr_ps.tile([P, 4, P], BF16, tag="tr")
                nc.tensor.transpose(warm_ps[:, 0, :], identB, identB)

                # ---------------- Phase 1: transpose x (already bf16) --------
                for half in range(2):
                    ps = tr_ps.tile([P, 4, P], BF16, tag="tr")
                    for i in range(4):
                        m = half * 4 + i
                        nc.tensor.transpose(ps[:, i, :], x_bf[:, m, :], identB)
                    nc.scalar.copy(xT_bf[:, half * 4:half * 4 + 4, :], ps)

                # Transpose adj bands 0 and 1 before phase 2 so the PE has
                # streaming work while the weight DMAs land.
                early_at = {}
                for m in range(2):
                    at = adjT_pool.tile([P, MT, P], BF16)
                    for half in range(2):
                        ps = tr_ps.tile([P, 4, P], BF16, tag="tr")
                        for i in range(4):
                            k = half * 4 + i
                            nc.tensor.transpose(
                                ps[:, i, :],
                                adj_tiles[m][:, k * P:(k + 1) * P], identB)
                        nc.scalar.copy(at[:, half * 4:half * 4 + 4, :], ps)
                    early_at[m] = at

                # ---------------- Phase 2: t = x@W_g + b_g (bf16) ------------
                for m in range(MT):
                    tp = mm_ps.tile([P, CO], F32, tag="mm256")
                    nc.tensor.matmul(tp, xT_bf[:, m, :], wg_sb,
                                     start=True, stop=True)
                    nc.vector.tensor_add(t_sb[:, m, :], tp, bg_bc)

                # ---------------- Phase 3: per-chunk gcn + LN1 + qkv ---------
                def emit_hT_qkv(mpair):
                    """h^T transposes + q/k/v projections for a chunk pair.
                    Emitted LAGGED behind the LN1 chain so the PE never
                    head-of-line blocks on DVE statistics."""
                    m0 = mpair[0]
                    psl = slice(m0 * P, (m0 + 2) * P)
                    ps = htr_ps.tile([P, 2, 2 * P], BF16, tag="htr")
                    for kc in range(2):
                        for j, mm in enumerate(mpair):
                            nc.tensor.transpose(
                                ps[:, kc, j * P:(j + 1) * P],
                                h_bf[:, mm, kc * P:(kc + 1) * P], identB)
                    nc.scalar.copy(hT_sb[:, :, psl], ps)
                    qkp = qk_ps.tile([P, 4, 2 * P], F32, tag="qk")
                    for oc in range(2):
                        nc.tensor.matmul(
                            qkp[:, oc, :], bq_row[:, oc * P:(oc + 1) * P],
                            ones1[:, 0:2 * P], start=True, stop=False,
                            skip_group_check=True)
                        for kc in range(2):
                            nc.tensor.matmul(
                                qkp[:, oc, :], wq_sb[:, kc, oc * P:(oc + 1) * P],
                                hT_sb[:, kc, psl],
                                start=False, stop=(kc == 1),
                                skip_group_check=True)
                    for oc in range(2):
                        for kc in range(2):
                            nc.tensor.matmul(
                                qkp[:, 2 + oc, :],
                                wk_sb[:, kc, oc * P:(oc + 1) * P],
                                hT_sb[:, kc, psl],
                                start=(kc == 0), stop=(kc == 1),
                                skip_group_check=True)
                    nc.scalar.copy(qT_sb[:, :, psl], qkp[:, 0:2, :])
                    nc.scalar.copy(kT_sb[:, :, psl], qkp[:, 2:4, :])
                    for mm in mpair:
                        msl = slice(mm * P, (mm + 1) * P)
                        vp = mm_ps.tile([P, CO], F32, tag="mm256")
                        for kc in range(2):
                            nc.tensor.matmul(vp, hT_sb[:, kc, msl],
                                             wv_sb[:, kc, :],
                                             start=(kc == 0), stop=(kc == 1))
                        nc.scalar.copy(v_sb[:, mm, :], vp)

                # adj bands arrive bf16 straight from DRAM, 3 ahead.
                ab_tiles = {0: adj_tiles[0], 1: adj_tiles[1], 2: adj_tiles[2]}
                pending_pair = None
                for m in range(MT):
                    if m + 3 < MT:
                        ab = adj_pool.tile([P, N], BF16, tag="ab")
                        nc.sync.dma_start(ab, adj_r[:, m + 3, :])
                        ab_tiles[m + 3] = ab
                    if m in early_at:
                        at = early_at[m]
                    else:
                        ab_bf = ab_tiles[m]
                        at = adjT_pool.tile([P, MT, P], BF16)
                        for half in range(2):
                            ps = tr_ps.tile([P, 4, P], BF16, tag="tr")
                            for i in range(4):
                                k = half * 4 + i
                                nc.tensor.transpose(
                                    ps[:, i, :], ab_bf[:, k * P:(k + 1) * P],
                                    identB)
                            nc.scalar.copy(at[:, half * 4:half * 4 + 4, :], ps)
                    # identity path for this chunk
                    ip = mm_ps.tile([P, CO], F32, tag="mm256")
                    nc.tensor.matmul(ip, xT_bf[:, m, :], wit_sb,
                                     start=True, stop=True)
                    id_sb = stemp.tile([P, CO], F32, tag="id_sb")
                    nc.vector.tensor_add(id_sb, ip, bit_bc)
                    # gcn chunk
                    gp = mm_ps.tile([P, CO], F32, tag="mm256")
                    for k in range(MT):
                        nc.tensor.matmul(gp, at[:, k, :], t_sb[:, k, :],
                                         start=(k == 0), stop=(k == MT - 1))
                    # s = identity + relu(gcn)
                    nc.vector.scalar_tensor_tensor(
                        out=s_all[:, m, :], in0=gp, scalar=0.0,
                        in1=id_sb, op0=ALU.max, op1=ALU.add)
                    stats = stemp.tile([P, 6], F32, tag="ln_stats")
                    nc.vector.bn_stats(out=stats, in_=s_all[:, m, :])
                    nc.vector.bn_aggr(out=mv_all[:, m, :], in_=stats)
                    if m % 2 == 1:
                        # rsqrt for the (m-1, m) pair; normalize both; emit
                        # the PREVIOUS pair's hT/qkv (two pairs of lag so the
                        # PE never waits on this DVE chain).
                        _rsqrt_dve(nc, stemp, mv_all[:, m - 1:m + 1, 1],
                                   rstd_all[:, m - 1:m + 1], consts, 2, "a",
                                   newton=1)
                        for mm in (m - 1, m):
                            nc.vector.tensor_scalar(
                                out=h_sb[:, mm, :], in0=s_all[:, mm, :],
                                scalar1=mv_all[:, mm, 0:1],
                                scalar2=rstd_all[:, mm:mm + 1],
                                op0=ALU.subtract, op1=ALU.mult)
                            nc.vector.tensor_copy(h_bf[:, mm, :],
                                                  h_sb[:, mm, :])
                        if pending_pair is not None:
                            emit_hT_qkv(pending_pair)
                        pending_pair = (m - 1, m)
                emit_hT_qkv(pending_pair)

            # ---------------- Phase 5: attention (group-pipelined) ----------
            # groups: (qh, g) in order; scores+exp of group i interleave with
            # attnV+den of group i-1 (one full group of lag).
            groups = [(qh, g) for qh in range(2) for g in range(2)]
            group_ex = {}   # gi -> list of 16 ex APs (bf16 views), slot order
            with ExitStack() as att:
                sc_ps = att.enter_context(
                    tc.tile_pool(name="sc_ps", bufs=2, space="PSUM"))
                acc_ps = att.enter_context(
                    tc.tile_pool(name="acc_ps", bufs=1, space="PSUM"))
                proj_ps = att.enter_context(
                    tc.tile_pool(name="proj_ps", bufs=2, space="PSUM"))

                def emit_scores_exp(gi, k):
                    """4 score matmuls (all 4 heads of the group, 4 row
                    groups co-issued), then the two exp tiles (ScalarE tp0,
                    DVE tp1 on most chunks)."""
                    qh, g = groups[gi]
                    qsl = slice(qh * 512, (qh + 1) * 512)
                    scs = []
                    for tp in range(2):
                        sc = sc_ps.tile([P, 1024], F32, tag="sc")
                        scs.append(sc)
                    for tp in range(2):
                        for j2 in range(2):
                            hh = 4 * g + 2 * tp + j2   # global head
                            bp = 32 * (hh % 4)
                            nc.tensor.matmul(
                                scs[tp][:, j2 * 512:(j2 + 1) * 512],
                                kT_sb[bp:bp + 32, g, k * P:(k + 1) * P],
                                qT_sb[bp:bp + 32, g, qsl],
                                start=True, stop=True,
                                tile_position=(bp, 0))
                    for tp in range(2):
                        if tp == 1 and k in DVE_EXP_KS:
                            exi = expT_pool.tile([P, 1024], I16, tag="exi")
                            nc.vector.tensor_scalar(
                                out=exi, in0=scs[tp], scalar1=EXP_A,
                                scalar2=EXP_B, op0=ALU.mult, op1=ALU.add)
                            group_ex[gi].append(exi.bitcast(BF16))
                        else:
                            ex = expT_pool.tile([P, 1024], BF16, tag="ex")
                            nc.scalar.activation(ex, scs[tp], AF.Exp,
                                                 scale=SCALE)
                            group_ex[gi].append(ex)

                def emit_avden(gi, k, acc):
                    """attn@V then denominators for chunk k (both head
                    pairs): two rounds of 4 matmuls, each round covering all
                    4 column groups so they run concurrently."""
                    _, g = groups[gi]
                    outb, denb = acc
                    for tp in range(2):
                        exs = group_ex[gi][2 * k + tp]
                        for j2 in range(2):
                            hh = 4 * g + 2 * tp + j2
                            cp = 32 * (hh % 4)
                            esl = slice(j2 * 512, (j2 + 1) * 512)
                            nc.tensor.matmul(
                                outb[cp:cp + 32, :],
                                v_sb[:, k, hh * D:(hh + 1) * D],
                                exs[:, esl],
                                start=(k == 0), stop=(k == MT - 1),
                                tile_position=(0, cp),
                                skip_group_check=True)
                    for tp in range(2):
                        exs = group_ex[gi][2 * k + tp]
                        for j2 in range(2):
                            hs = 4 * g + 2 * tp + (1 - j2)  # swapped cols
                            cps = 32 * (hs % 4)
                            esls = slice((1 - j2) * 512, (2 - j2) * 512)
                            nc.tensor.matmul(
                                denb[cps:cps + 32, :],
                                ones_sb,
                                exs[:, esls],
                                start=(k == 0), stop=(k == MT - 1),
                                tile_position=(0, cps),
                                skip_group_check=True)

                def finish_group(gi, acc):
                    qh, g = groups[gi]
                    qsl = slice(qh * 512, (qh + 1) * 512)
                    outb, denb = acc
                    rec = stemp.tile([P, 512], F32, tag="rec")
                    nc.vector.reciprocal_approx_fast(out=rec, in_=denb)
                    nc.vector.tensor_mul(outT_sb[:, g, qsl], outb, rec)

                def proj_ln2_store(qh):
                    """Projection + residual + LN2 + DMA for 4 chunks."""
                    s2s = []
                    mv2 = ptemp.tile([P, 4, 2], F32, tag="mv2")
                    for i in range(4):
                        m = qh * 4 + i
                        pp = proj_ps.tile([P, CO], F32, tag="proj")
                        nc.tensor.matmul(pp, ones1[:, 0:P], bb2_row,
                                         start=True, stop=False)
                        for cc in range(2):
                            nc.tensor.matmul(
                                pp, outT_sb[:, cc, m * P:(m + 1) * P],
                                wo_sb[:, cc, :],
                                start=False, stop=(cc == 1))
                        # s2 = h*g1 + proj + bb2  (bb2 already in psum)
                        s2 = ptemp.tile([P, CO], F32, tag=f"s2_{i}")
                        if trivial1:
                            nc.vector.tensor_add(s2, pp, h_sb[:, m, :])
                        else:
                            nc.vector.tensor_mul(s2, h_sb[:, m, :], g1_bc)
                            nc.vector.tensor_add(s2, s2, pp)
                        stats = ptemp.tile([P, 6], F32, tag="ln_stats2")
                        nc.vector.bn_stats(out=stats, in_=s2)
                        nc.vector.bn_aggr(out=mv2[:, i, :], in_=stats)
                        s2s.append(s2)
                    rstd2 = ptemp.tile([P, 4], F32, tag="rstd2")
                    _rsqrt_dve(nc, ptemp, mv2[:, :, 1], rstd2, consts, 4, "b")
                    for i in range(4):
                        m = qh * 4 + i
                        yt = ytile_pool.tile([P, CO], F32)
                        nc.vector.tensor_scalar(
                            out=yt, in0=s2s[i],
                            scalar1=mv2[:, i, 0:1], scalar2=rstd2[:, i:i + 1],
                            op0=ALU.subtract, op1=ALU.mult)
                        if not trivial2:
                            nc.vector.tensor_mul(yt, yt, g2_bc)
                            nc.vector.tensor_add(yt, yt, be2_bc)
                        nc.sync.dma_start(
                            out_d[:].rearrange("(mt p) c -> p mt c", p=P)[:, m, :],
                            yt)

                # Flat slot schedule over all (group, chunk) pairs with a
                # short uniform lag: attnV+den of slot i-LAG interleave with
                # scores+exp of slot i. LAG=3 chunks is plenty of slack for
                # the exp engines, and shrinks the drain tail after the last
                # scores from a full group (16 slots) to LAG slots.
                LAG = 3
                accs = {}
                flat = [(gi, k) for gi in range(len(groups))
                        for k in range(MT)]

                def emit_lagged(j):
                    gj, kj = flat[j]
                    emit_avden(gj, kj, accs[gj])
                    if kj == MT - 1:
                        finish_group(gj, accs[gj])
                        if gj == 1:
                            # outT for qh=0 complete -> drain it while the
                            # qh=1 groups stream.
                            proj_ln2_store(0)

                for idx, (gi, k) in enumerate(flat):
                    if k == 0:
                        group_ex[gi] = []
                        at2 = acc_ps.tile([P, 2, 512], F32, tag="acc")
                        accs[gi] = (at2[:, 0, :], at2[:, 1, :])
                    emit_scores_exp(gi, k)
                    if idx >= LAG:
                        emit_lagged(idx - LAG)
                for j in range(len(flat) - LAG, len(flat)):
                    emit_lagged(j)
                proj_ln2_store(1)

    nc.finalize()
    return nc


_CACHE = {}


def _get_nc(trivial1, trivial2):
    key = (trivial1, trivial2)
    if key not in _CACHE:
        _CACHE[key] = build_bass(*key)
    return _CACHE[key]


def _prep_host(inputs):
    """Fold LN1 affine + attention biases into weights on the host (fp32),
    cast weights to bf16, and return (shared input map, flags)."""
    import ml_dtypes

    BF = ml_dtypes.bfloat16
    f = {k: np.ascontiguousarray(np.asarray(v, np.float32))
         for k, v in inputs.items()}
    g1, be1 = f["g1"], f["beta1"]
    g2, be2 = f["g2"], f["beta2"]
    wq = g1[:, None] * f["W_q"]
    bq = f["b_q"] + be1 @ f["W_q"]
    wk = g1[:, None] * f["W_k"]
    wv = g1[:, None] * f["W_v"]
    bv = f["b_v"] + be1 @ f["W_v"]
    bb2 = be1 + f["b_o"] + bv @ f["W_o"]

    trivial1 = bool(np.all(g1 == 1.0))
    trivial2 = bool(np.all(g2 == 1.0) and np.all(be2 == 0.0))

    def bf(a):
        return np.ascontiguousarray(a.astype(BF))

    shared = {
        "wit": bf(f["W_it"]), "wg": bf(f["W_g"]),
        "wq": bf(wq), "wk": bf(wk), "wv": bf(wv), "wo": bf(f["W_o"]),
        "bit": f["b_it"], "bg": f["b_g"],
        "bq": bf(bq), "bb2": bf(bb2),
    }
    if not trivial1:
        shared["g1v"] = g1
    if not trivial2:
        shared["g2v"] = g2
        shared["be2v"] = be2
    return shared, trivial1, trivial2


def run(inputs, trace=False):
    shared, trivial1, trivial2 = _prep_host(inputs)
    nc = _get_nc(trivial1, trivial2)
    import ml_dtypes
    x = np.ascontiguousarray(np.asarray(inputs["x"]).astype(ml_dtypes.bfloat16))
    adj = np.ascontiguousarray(
        np.asarray(inputs["adj"]).astype(ml_dtypes.bfloat16))
    in_maps = []
    for b in range(NCORES):
        m = dict(shared)
        m["x"] = x[b]
        m["adj"] = adj[b]
        in_maps.append(m)
    res = run_bass_kernel_spmd(nc, in_maps, core_ids=list(range(NCORES)),
                               trace=trace)
    out = np.stack([res.results[b]["out"] for b in range(NCORES)], axis=0)
    return out, res


def kernel(**inputs):
    out, _ = run(inputs, trace=False)
    return out


# revision 29
# speedup vs baseline: 1.0125x; 1.0125x over previous
"""AttentionGCNLayer Trainium2 kernel (final).

Per-sample computation (B=8 samples -> 8 NeuronCores, data-parallel):
  identity = x @ W_it + b_it
  gcn      = relu(adj @ (x @ W_g + b_g))
  h        = LN1(identity + gcn)
  attn     = MHSA(h)  (8 heads, D=32)
  out      = LN2(h + attn)

Design:
  - Host-side weight folding: LN1's gamma folds into W_q/W_k/W_v rows; the
    k-bias drops entirely (softmax is invariant to per-query shifts); the
    v-bias folds into the output-projection bias (softmax rows sum to 1).
    All weights pre-cast to bf16 on the host -> every matmul/transpose is
    single-pass bf16.
  - Softmax exp split across ScalarE (table exp) and VectorE (Schraudolph
    bit-trick: i16 = round(score * 128*scale/ln2 + const), bitcast i16->bf16
    ~= exp(score*scale)). Softmax normalization cancels the approximation's
    bias; end-to-end error ~2.7e-3 (tolerance 2e-2).
  - The per-chunk pre-phase loop emits hT/qkv for chunks PAIRED and LAGGED
    behind the LN1 DVE chain, so the PE never head-of-line blocks on the
    LN statistics; LN1 rsqrt runs one Newton step on chunk pairs.
  - Attention runs a flat slot schedule over all (token-half, head-group,
    k-chunk) slots: attnV+den of slot i-3 interleave with scores+exp of
    slot i, so the PE always has streaming work regardless of exp latency
    and the post-scores drain tail is only 3 slots.
  - adj and x ship as bf16 from the host (validated: no measurable error
    change), halving the dominant DMA traffic and removing all on-device
    input casts.
  - Projection + LN2 + store for the first token half drain while the
    second half's attention streams; the second half is the only tail.
"""

import sys

sys.path.insert(0, "/opt/trn_rl_repo")

import numpy as np

import concourse.bass as bass
import concourse.tile as tile
from concourse import bacc, mybir
from concourse.bass_utils import run_bass_kernel_spmd
from concourse.masks import make_identity

F32 = mybir.dt.float32
BF16 = mybir.dt.bfloat16
I16 = mybir.dt.int16
I32 = mybir.dt.int32
AF = mybir.ActivationFunctionType
ALU = mybir.AluOpType

B, N, CI, CO, H, D = 8, 1024, 128, 256, 8, 32
P = 128
MT = N // P  # 8 token chunks
EPS = 1e-5
SCALE = float(1.0 / np.sqrt(np.float32(D)))
NCORES = 8
MAGIC_P1 = 0x5F3759DF + 1  # quake rsqrt magic + 1 (for the ~t + (M+1) form)

# Schraudolph constants: bf16 bit pattern of exp(scale*x) via int16 affine.
EXP_A = float(SCALE * 128.0 / np.log(2.0))
EXP_B = float(127 * 128 - 9.0)

# which exp slots go to the DVE: (tp == 1) and k in the group's set. Groups
# 2-3 shed DVE exp tiles because the DVE also runs the overlapped proj/LN2
# and reciprocal chains there, and a delayed exp tile stalls the PE via the
# scores-buffer WAR.
DVE_EXP_KS_BY_GROUP = {
    0: (0, 1, 2, 4, 5, 6),
    1: (0, 1, 2, 4, 5, 6),
    2: (0, 4, 5, 6),
    3: (0, 4, 5, 6),
}


def _rsqrt_dve(nc, pool, var_ap, out_ap, consts, n, tag, newton=2):
    """out = 1/sqrt(var + eps) on VectorE only, batched over [128, n].

    Quake bit-trick seed + `newton` Newton iterations (1 iter ~0.17% rel
    err, 2 iters ~5e-6). Keeps ScalarE free of Ln/Sqrt so its activation
    table never switches off Exp.
    """
    eps_sb, sh1_i, neg1_i, magic_i = consts
    xe = pool.tile([P, n], F32, tag=f"rs_xe{tag}")
    nc.vector.tensor_scalar_add(xe, var_ap, eps_sb)
    y = pool.tile([P, n], F32, tag=f"rs_y{tag}")
    ti = pool.tile([P, n], I32, tag=f"rs_ti{tag}")
    # ~(x >> 1)
    nc.vector.tensor_scalar(
        out=ti, in0=xe.bitcast(I32), scalar1=sh1_i, scalar2=neg1_i,
        op0=ALU.logical_shift_right, op1=ALU.bitwise_xor)
    # + (MAGIC+1)  ==  MAGIC - (x >> 1)
    nc.vector.tensor_tensor(
        out=y.bitcast(I32), in0=ti, in1=magic_i.to_broadcast((P, n)), op=ALU.add)
    h = pool.tile([P, n], F32, tag=f"rs_h{tag}")
    nc.vector.tensor_scalar_mul(h, xe, 0.5)
    t2 = pool.tile([P, n], F32, tag=f"rs_t2{tag}")
    for _ in range(newton):
        nc.vector.tensor_mul(t2, y, y)
        nc.vector.tensor_mul(t2, t2, h)
        nc.vector.tensor_scalar(
            out=t2, in0=t2, scalar1=-1.0, scalar2=1.5, op0=ALU.mult, op1=ALU.add)
        nc.vector.tensor_mul(y, y, t2)
    nc.vector.tensor_copy(out_ap, y)


def build_bass(trivial1=True, trivial2=True):
    nc = bacc.Bacc()

    x_d = nc.dram_tensor("x", (N, CI), BF16, kind="ExternalInput")
    adj_d = nc.dram_tensor("adj", (N, N), BF16, kind="ExternalInput")
    wit_d = nc.dram_tensor("wit", (CI, CO), BF16, kind="ExternalInput")
    wg_d = nc.dram_tensor("wg", (CI, CO), BF16, kind="ExternalInput")
    wq_d = nc.dram_tensor("wq", (CO, CO), BF16, kind="ExternalInput")
    wk_d = nc.dram_tensor("wk", (CO, CO), BF16, kind="ExternalInput")
    wv_d = nc.dram_tensor("wv", (CO, CO), BF16, kind="ExternalInput")
    wo_d = nc.dram_tensor("wo", (CO, CO), BF16, kind="ExternalInput")
    bit_d = nc.dram_tensor("bit", (CO,), F32, kind="ExternalInput")
    bg_d = nc.dram_tensor("bg", (CO,), F32, kind="ExternalInput")
    bq_d = nc.dram_tensor("bq", (CO,), BF16, kind="ExternalInput")
    bb2_d = nc.dram_tensor("bb2", (CO,), BF16, kind="ExternalInput")
    if not trivial1:
        g1_d = nc.dram_tensor("g1v", (CO,), F32, kind="ExternalInput")
    if not trivial2:
        g2_d = nc.dram_tensor("g2v", (CO,), F32, kind="ExternalInput")
        be2_d = nc.dram_tensor("be2v", (CO,), F32, kind="ExternalInput")
    out_d = nc.dram_tensor("out", (N, CO), F32, kind="ExternalOutput")

    with tile.TileContext(nc) as tc:
        from contextlib import ExitStack

        with ExitStack() as ctx:
            singles = ctx.enter_context(tc.tile_pool(name="singles", bufs=1))
            stemp = ctx.enter_context(tc.tile_pool(name="stemp", bufs=3))
            ptemp = ctx.enter_context(tc.tile_pool(name="ptemp", bufs=5))
            adj_pool = ctx.enter_context(tc.tile_pool(name="adj", bufs=3))
            adjb_pool = ctx.enter_context(tc.tile_pool(name="adjb", bufs=3))
            adjT_pool = ctx.enter_context(tc.tile_pool(name="adjT", bufs=3))
            expT_pool = ctx.enter_context(tc.tile_pool(name="expT", bufs=20))
            ytile_pool = ctx.enter_context(tc.tile_pool(name="ytile", bufs=2))

            # ---------------- Phase -1: identity + big input DMAs first -----
            identB = singles.tile([P, P], BF16)
            make_identity(nc, identB)
            adj_r = adj_d[:].rearrange("(mt p) k -> p mt k", p=P)
            x_sb = singles.tile([P, MT, CI], BF16)
            nc.gpsimd.dma_start(x_sb, x_d[:].rearrange("(mt p) c -> p mt c", p=P))
            adj_tiles = []
            for m, eng in zip(range(3), (nc.sync, nc.scalar, nc.gpsimd)):
                ab = adj_pool.tile([P, N], BF16, tag="ab")
                eng.dma_start(ab, adj_r[:, m, :])
                adj_tiles.append(ab)

            # ---------------- Phase 0: constants / weights ----------------
            ones_sb = singles.tile([P, D], BF16)
            nc.vector.memset(ones_sb, 1.0)
            ones1 = singles.tile([1, 512], BF16)
            nc.vector.memset(ones1, 1.0)
            eps_sb = singles.tile([P, 1], F32)
            nc.vector.memset(eps_sb, EPS)
            sh1_i = singles.tile([P, 1], I32)
            nc.vector.memset(sh1_i, 1)
            neg1_i = singles.tile([P, 1], I32)
            nc.vector.memset(neg1_i, -1)
            magic_i = singles.tile([P, 1], I32)
            nc.vector.memset(magic_i, MAGIC_P1)
            consts = (eps_sb, sh1_i, neg1_i, magic_i)

            def load_w2(dram, name):  # [256,256] -> [128, 2, 256]
                t = singles.tile([P, 2, CO], BF16, tag=f"w2_{name}")
                nc.gpsimd.dma_start(
                    t, dram[:].rearrange("(ko ki) n -> ki ko n", ki=P))
                return t

            def load_row(dram, name):  # [256] -> [1, 256] single-partition row
                t = singles.tile([1, CO], BF16, tag=f"row_{name}")
                nc.gpsimd.dma_start(t, dram[:].rearrange("(a c) -> a c", a=1))
                return t

            def load_bc(dram, name):  # broadcast along partitions: [128, 256]
                t = singles.tile([P, CO], F32, tag=f"bc_{name}")
                src = dram[:]
                bcast = bass.AP(tensor=src.tensor, offset=src.offset,
                                ap=[[0, P]] + list(src.ap))
                nc.gpsimd.dma_start(out=t, in_=bcast)
                return t

            # gpsimd DMA queue ordered by first use: phase-2/gcn inputs
            # first, attention weights after, output-side last.
            wg_sb = singles.tile([P, CO], BF16)
            nc.gpsimd.dma_start(wg_sb, wg_d[:])
            bg_bc = load_bc(bg_d, "bg")
            wit_sb = singles.tile([P, CO], BF16)
            nc.gpsimd.dma_start(wit_sb, wit_d[:])
            bit_bc = load_bc(bit_d, "bit")
            wq_sb = load_w2(wq_d, "wq")
            wk_sb = load_w2(wk_d, "wk")
            wv_sb = load_w2(wv_d, "wv")
            bq_row = load_row(bq_d, "bq")
            wo_sb = load_w2(wo_d, "wo")
            bb2_row = load_row(bb2_d, "bb2")
            if not trivial1:
                g1_bc = load_bc(g1_d, "g1")
            if not trivial2:
                g2_bc = load_bc(g2_d, "g2")
                be2_bc = load_bc(be2_d, "be2")

            # persistent activations
            x_bf = x_sb                                  # already bf16
            xT_bf = singles.tile([P, MT, P], BF16)       # x^T  [ci, m] bf16
            t_sb = singles.tile([P, MT, CO], BF16)       # x@W_g + b_g [tok, c]
            s_all = singles.tile([P, MT, CO], F32)       # pre-LN1 residual
            h_sb = singles.tile([P, MT, CO], F32)        # normalized hhat
            h_bf = singles.tile([P, MT, CO], BF16)       # hhat bf16 (for h^T)
            mv_all = singles.tile([P, MT, 2], F32)       # LN1 mean/var
            rstd_all = singles.tile([P, MT], F32)        # LN1 rstd
            hT_sb = singles.tile([P, 2, N], BF16)        # hhat^T  [c, tok]
            qT_sb = singles.tile([P, 2, N], BF16)        # q^T     [c, tok]
            kT_sb = singles.tile([P, 2, N], BF16)        # k^T     [c, tok]
            v_sb = singles.tile([P, MT, CO], BF16)       # v       [tok, c]
            outT_sb = singles.tile([P, 2, N], BF16)      # attn-out^T [c, tok]

            with ExitStack() as pre:
                tr_ps = pre.enter_context(
                    tc.tile_pool(name="tr_ps", bufs=2, space="PSUM"))
                htr_ps = pre.enter_context(
                    tc.tile_pool(name="htr_ps", bufs=1, space="PSUM"))
                mm_ps = pre.enter_context(
                    tc.tile_pool(name="mm_ps", bufs=2, space="PSUM"))
                qk_ps = pre.enter_context(
                    tc.tile_pool(name="qk_ps", bufs=1, space="PSUM"))

                # Warm-up transpose so PE observes the gpsimd sem early.
                warm_ps = tr_ps.tile([P, 4, P], BF16, tag="tr")
                nc.tensor.transpose(warm_ps[:, 0, :], identB, identB)

                # ---------------- Phase 1: transpose x (already bf16) --------
                for half in range(2):
                    ps = tr_ps.tile([P, 4, P], BF16, tag="tr")
                    for i in range(4):
                        m = half * 4 + i
                        nc.tensor.transpose(ps[:, i, :], x_bf[:, m, :], identB)
                    nc.scalar.copy(xT_bf[:, half * 4:half * 4 + 4, :], ps)

                # Transpose adj bands 0 and 1 before phase 2 so the PE has
                # streaming work while the weight DMAs land.
                early_at = {}
                for m in range(2):
                    at = adjT_pool.tile([P, MT, P], BF16)
                    for half in range(2):
                        ps = tr_ps.tile([P, 4, P], BF16, tag="tr")
                        for i in range(4):
                            k = half * 4 + i
                            nc.tensor.transpose(
                                ps[:, i, :],
                                adj_tiles[m][:, k * P:(k + 1) * P], identB)
                        nc.scalar.copy(at[:, half * 4:half * 4 + 4, :], ps)
                    early_at[m] = at

                # ---------------- Phase 2: t = x@W_g + b_g (bf16) ------------
                for m in range(MT):
                    tp = mm_ps.tile([P, CO], F32, tag="mm256")
                    nc.tensor.matmul(tp, xT_bf[:, m, :], wg_sb,
                                     start=True, stop=True)
                    nc.vector.tensor_add(t_sb[:, m, :], tp, bg_bc)

                # ---------------- Phase 3: per-chunk gcn + LN1 + qkv ---------
                def emit_hT_qkv(mpair):
                    """h^T transposes + q/k/v projections for a chunk pair.
                    Emitted LAGGED behind the LN1 chain so the PE never
                    head-of-line blocks on DVE statistics."""
                    m0 = mpair[0]
                    psl = slice(m0 * P, (m0 + 2) * P)
                    ps = htr_ps.tile([P, 2, 2 * P], BF16, tag="htr")
                    for kc in range(2):
                        for j, mm in enumerate(mpair):
                            nc.tensor.transpose(
                                ps[:, kc, j * P:(j + 1) * P],
                                h_bf[:, mm, kc * P:(kc + 1) * P], identB)
                    nc.scalar.copy(hT_sb[:, :, psl], ps)
                    qkp = qk_ps.tile([P, 4, 2 * P], F32, tag="qk")
                    for oc in range(2):
                        nc.tensor.matmul(
                            qkp[:, oc, :], bq_row[:, oc * P:(oc + 1) * P],
                            ones1[:, 0:2 * P], start=True, stop=False,
                            skip_group_check=True)
                        for kc in range(2):
                            nc.tensor.matmul(
                                qkp[:, oc, :], wq_sb[:, kc, oc * P:(oc + 1) * P],
                                hT_sb[:, kc, psl],
                                start=False, stop=(kc == 1),
                                skip_group_check=True)
                    for oc in range(2):
                        for kc in range(2):
                            nc.tensor.matmul(
                                qkp[:, 2 + oc, :],
                                wk_sb[:, kc, oc * P:(oc + 1) * P],
                                hT_sb[:, kc, psl],
                                start=(kc == 0), stop=(kc == 1),
                                skip_group_check=True)
                    nc.scalar.copy(qT_sb[:, :, psl], qkp[:, 0:2, :])
                    nc.scalar.copy(kT_sb[:, :, psl], qkp[:, 2:4, :])
                    for mm in mpair:
                        msl = slice(mm * P, (mm + 1) * P)
                        vp = mm_ps.tile([P, CO], F32, tag="mm256")
                        for kc in range(2):
                            nc.tensor.matmul(vp, hT_sb[:, kc, msl],
                                             wv_sb[:, kc, :],
                                             start=(kc == 0), stop=(kc == 1))
                        nc.scalar.copy(v_sb[:, mm, :], vp)

                # adj bands arrive bf16 straight from DRAM, 3 ahead.
                ab_tiles = {0: adj_tiles[0], 1: adj_tiles[1], 2: adj_tiles[2]}
                pending_pair = None
                for m in range(MT):
                    if m + 3 < MT:
                        ab = adj_pool.tile([P, N], BF16, tag="ab")
                        nc.sync.dma_start(ab, adj_r[:, m + 3, :])
                        ab_tiles[m + 3] = ab
                    if m in early_at:
                        at = early_at[m]
                    else:
                        ab_bf = ab_tiles[m]
                        at = adjT_pool.tile([P, MT, P], BF16)
                        for half in range(2):
                            ps = tr_ps.tile([P, 4, P], BF16, tag="tr")
                            for i in range(4):
                                k = half * 4 + i
                                nc.tensor.transpose(
                                    ps[:, i, :], ab_bf[:, k * P:(k + 1) * P],
                                    identB)
                            nc.scalar.copy(at[:, half * 4:half * 4 + 4, :], ps)
                    # identity path for this chunk
                    ip = mm_ps.tile([P, CO], F32, tag="mm256")
                    nc.tensor.matmul(ip, xT_bf[:, m, :], wit_sb,
                                     start=True, stop=True)
                    id_sb = stemp.tile([P, CO], F32, tag="id_sb")
                    nc.vector.tensor_add(id_sb, ip, bit_bc)
                    # gcn chunk
                    gp = mm_ps.tile([P, CO], F32, tag="mm256")
                    for k in range(MT):
                        nc.tensor.matmul(gp, at[:, k, :], t_sb[:, k, :],
                                         start=(k == 0), stop=(k == MT - 1))
                    # s = identity + relu(gcn)
                    nc.vector.scalar_tensor_tensor(
                        out=s_all[:, m, :], in0=gp, scalar=0.0,
                        in1=id_sb, op0=ALU.max, op1=ALU.add)
                    stats = stemp.tile([P, 6], F32, tag="ln_stats")
                    nc.vector.bn_stats(out=stats, in_=s_all[:, m, :])
                    nc.vector.bn_aggr(out=mv_all[:, m, :], in_=stats)
                    if m % 2 == 1:
                        # rsqrt for the (m-1, m) pair; normalize both; emit
                        # the PREVIOUS pair's hT/qkv (two pairs of lag so the
                        # PE never waits on this DVE chain).
                        _rsqrt_dve(nc, stemp, mv_all[:, m - 1:m + 1, 1],
                                   rstd_all[:, m - 1:m + 1], consts, 2, "a",
                                   newton=1)
                        for mm in (m - 1, m):
                            nc.vector.tensor_scalar(
                                out=h_sb[:, mm, :], in0=s_all[:, mm, :],
                                scalar1=mv_all[:, mm, 0:1],
                                scalar2=rstd_all[:, mm:mm + 1],
                                op0=ALU.subtract, op1=ALU.mult)
                            nc.vector.tensor_copy(h_bf[:, mm, :],
                                                  h_sb[:, mm, :])
                        if pending_pair is not None:
                            emit_hT_qkv(pending_pair)
                        pending_pair = (m - 1, m)
                emit_hT_qkv(pending_pair)

            # ---------------- Phase 5: attention (group-pipelined) ----------
            # groups: (qh, g) in order; scores+exp of group i interleave with
            # attnV+den of group i-1 (one full group of lag).
            groups = [(qh, g) for qh in range(2) for g in range(2)]
            group_ex = {}   # gi -> list of 16 ex APs (bf16 views), slot order
            with ExitStack() as att:
                sc_ps = att.enter_context(
                    tc.tile_pool(name="sc_ps", bufs=2, space="PSUM"))
                acc_ps = att.enter_context(
                    tc.tile_pool(name="acc_ps", bufs=1, space="PSUM"))
                proj_ps = att.enter_context(
                    tc.tile_pool(name="proj_ps", bufs=2, space="PSUM"))

                def emit_scores_exp(gi, k):
                    """4 score matmuls (all 4 heads of the group, 4 row
                    groups co-issued), then the two exp tiles (ScalarE tp0,
                    DVE tp1 on most chunks)."""
                    qh, g = groups[gi]
                    qsl = slice(qh * 512, (qh + 1) * 512)
                    scs = []
                    for tp in range(2):
                        sc = sc_ps.tile([P, 1024], F32, tag="sc")
                        scs.append(sc)
                    for tp in range(2):
                        for j2 in range(2):
                            hh = 4 * g + 2 * tp + j2   # global head
                            bp = 32 * (hh % 4)
                            nc.tensor.matmul(
                                scs[tp][:, j2 * 512:(j2 + 1) * 512],
                                kT_sb[bp:bp + 32, g, k * P:(k + 1) * P],
                                qT_sb[bp:bp + 32, g, qsl],
                                start=True, stop=True,
                                tile_position=(bp, 0))
                    for tp in range(2):
                        if tp == 1 and k in DVE_EXP_KS_BY_GROUP[gi]:
                            exi = expT_pool.tile([P, 1024], I16, tag="exi")
                            nc.vector.tensor_scalar(
                                out=exi, in0=scs[tp], scalar1=EXP_A,
                                scalar2=EXP_B, op0=ALU.mult, op1=ALU.add)
                            group_ex[gi].append(exi.bitcast(BF16))
                        else:
                            ex = expT_pool.tile([P, 1024], BF16, tag="ex")
                            nc.scalar.activation(ex, scs[tp], AF.Exp,
                                                 scale=SCALE)
                            group_ex[gi].append(ex)

                def emit_avden(gi, k, acc):
                    """attn@V then denominators for chunk k (both head
                    pairs): two rounds of 4 matmuls, each round covering all
                    4 column groups so they run concurrently."""
                    _, g = groups[gi]
                    outb, denb = acc
                    for tp in range(2):
                        exs = group_ex[gi][2 * k + tp]
                        for j2 in range(2):
                            hh = 4 * g + 2 * tp + j2
                            cp = 32 * (hh % 4)
                            esl = slice(j2 * 512, (j2 + 1) * 512)
                            nc.tensor.matmul(
                                outb[cp:cp + 32, :],
                                v_sb[:, k, hh * D:(hh + 1) * D],
                                exs[:, esl],
                                start=(k == 0), stop=(k == MT - 1),
                                tile_position=(0, cp),
                                skip_group_check=True)
                    for tp in range(2):
                        exs = group_ex[gi][2 * k + tp]
                        for j2 in range(2):
                            hs = 4 * g + 2 * tp + (1 - j2)  # swapped cols
                            cps = 32 * (hs % 4)
                            esls = slice((1 - j2) * 512, (2 - j2) * 512)
                            nc.tensor.matmul(
                                denb[cps:cps + 32, :],
                                ones_sb,
                                exs[:, esls],
                                start=(k == 0), stop=(k == MT - 1),
                                tile_position=(0, cps),
                                skip_group_check=True)

                def finish_group(gi, acc):
                    qh, g = groups[gi]
                    qsl = slice(qh * 512, (qh + 1) * 512)
                    outb, denb = acc
                    rec = stemp.tile([P, 512], F32, tag="rec")
                    nc.vector.reciprocal_approx_fast(out=rec, in_=denb)
                    nc.vector.tensor_mul(outT_sb[:, g, qsl], outb, rec)

                def proj_ln2_store(qh):
                    """Projection + residual + LN2 + DMA for 4 chunks."""
                    s2s = []
                    mv2 = ptemp.tile([P, 4, 2], F32, tag="mv2")
                    for i in range(4):
                        m = qh * 4 + i
                        pp = proj_ps.tile([P, CO], F32, tag="proj")
                        nc.tensor.matmul(pp, ones1[:, 0:P], bb2_row,
                                         start=True, stop=False)
                        for cc in range(2):
                            nc.tensor.matmul(
                                pp, outT_sb[:, cc, m * P:(m + 1) * P],
                                wo_sb[:, cc, :],
                                start=False, stop=(cc == 1))
                        # s2 = h*g1 + proj + bb2  (bb2 already in psum)
                        s2 = ptemp.tile([P, CO], F32, tag=f"s2_{i}")
                        if trivial1:
                            nc.vector.tensor_add(s2, pp, h_sb[:, m, :])
                        else:
                            nc.vector.tensor_mul(s2, h_sb[:, m, :], g1_bc)
                            nc.vector.tensor_add(s2, s2, pp)
                        stats = ptemp.tile([P, 6], F32, tag="ln_stats2")
                        nc.vector.bn_stats(out=stats, in_=s2)
                        nc.vector.bn_aggr(out=mv2[:, i, :], in_=stats)
                        s2s.append(s2)
                    rstd2 = ptemp.tile([P, 4], F32, tag="rstd2")
                    _rsqrt_dve(nc, ptemp, mv2[:, :, 1], rstd2, consts, 4, "b")
                    for i in range(4):
                        m = qh * 4 + i
                        yt = ytile_pool.tile([P, CO], F32)
                        nc.vector.tensor_scalar(
                            out=yt, in0=s2s[i],
                            scalar1=mv2[:, i, 0:1], scalar2=rstd2[:, i:i + 1],
                            op0=ALU.subtract, op1=ALU.mult)
                        if not trivial2:
                            nc.vector.tensor_mul(yt, yt, g2_bc)
                            nc.vector.tensor_add(yt, yt, be2_bc)
                        nc.sync.dma_start(
                            out_d[:].rearrange("(mt p) c -> p mt c", p=P)[:, m, :],
                            yt)

                # Flat slot schedule over all (group, chunk) pairs with a
                # short uniform lag: attnV+den of slot i-LAG interleave with
                # scores+exp of slot i. LAG=3 chunks is plenty of slack for
                # the exp engines, and shrinks the drain tail after the last
                # scores from a full group (16 slots) to LAG slots.
                LAG = 3
                accs = {}
                flat = [(gi, k) for gi in range(len(groups))
                        for k in range(MT)]

                def emit_lagged(j):
                    gj, kj = flat[j]
                    emit_avden(gj, kj, accs[gj])
                    if kj == MT - 1:
                        finish_group(gj, accs[gj])
                        if gj == 1:
                            # outT for qh=0 complete -> drain it while the
                            # qh=1 groups stream.
                            proj_ln2_store(0)

                for idx, (gi, k) in enumerate(flat):
                    if k == 0:
                        group_ex[gi] = []
                        at2 = acc_ps.tile([P, 2, 512], F32, tag="acc")
                        accs[gi] = (at2[:, 0, :], at2[:, 1, :])
                    emit_scores_exp(gi, k)
                    if idx >= LAG:
                        emit_lagged(idx - LAG)
                for j in range(len(flat) - LAG, len(flat)):
                    emit_lagged(j)
                proj_ln2_store(1)

    nc.finalize()
    return nc


_CACHE = {}


def _get_nc(trivial1, trivial2):
    key = (trivial1, trivial2)
    if key not in _CACHE:
        _CACHE[key] = build_bass(*key)
    return _CACHE[key]


def _prep_host(inputs):
    """Fold LN1 affine + attention biases into weights on the host (fp32),
    cast weights to bf16, and return (shared input map, flags)."""
    import ml_dtypes

    BF = ml_dtypes.bfloat16
    f = {k: np.ascontiguousarray(np.asarray(v, np.float32))
         for k, v in inputs.items()}
    g1, be1 = f["g1"], f["beta1"]
    g2, be2 = f["g2"], f["beta2"]
    wq = g1[:, None] * f["W_q"]
    bq = f["b_q"] + be1 @ f["W_q"]
    wk = g1[:, None] * f["W_k"]
    wv = g1[:, None] * f["W_v"]
    bv = f["b_v"] + be1 @ f["W_v"]
    bb2 = be1 + f["b_o"] + bv @ f["W_o"]

    trivial1 = bool(np.all(g1 == 1.0))
    trivial2 = bool(np.all(g2 == 1.0) and np.all(be2 == 0.0))

    def bf(a):
        return np.ascontiguousarray(a.astype(BF))

    shared = {
        "wit": bf(f["W_it"]), "wg": bf(f["W_g"]),
        "wq": bf(wq), "wk": bf(wk), "wv": bf(wv), "wo": bf(f["W_o"]),
        "bit": f["b_it"], "bg": f["b_g"],
        "bq": bf(bq), "bb2": bf(bb2),
    }
    if not trivial1:
        shared["g1v"] = g1
    if not trivial2:
        shared["g2v"] = g2
        shared["be2v"] = be2
    return shared, trivial1, trivial2


def run(inputs, trace=False):
    shared, trivial1, trivial2 = _prep_host(inputs)
    nc = _get_nc(trivial1, trivial2)
    import ml_dtypes
    x = np.ascontiguousarray(np.asarray(inputs["x"]).astype(ml_dtypes.bfloat16))
    adj = np.ascontiguousarray(
        np.asarray(inputs["adj"]).astype(ml_dtypes.bfloat16))
    in_maps = []
    for b in range(NCORES):
        m = dict(shared)
        m["x"] = x[b]
        m["adj"] = adj[b]
        in_maps.append(m)
    res = run_bass_kernel_spmd(nc, in_maps, core_ids=list(range(NCORES)),
                               trace=trace)
    out = np.stack([res.results[b]["out"] for b in range(NCORES)], axis=0)
    return out, res


def kernel(**inputs):
    out, _ = run(inputs, trace=False)
    return out
